# revision 1
# baseline (speedup 1.0000x reference)
"""Multi-head causal attention (B=2, S=2048, D=1024, H=16, hd=64) on 8 TRN2
NeuronCores.

Sharding: tensor-parallel over heads — 2 heads per core. Each core computes
Q/K/V for its 2 heads over the full sequence, causal attention, and a partial
output projection (its 128 context features x Wo slice). Host sums the 8
partials (f16) and adds the bias.

Matmuls run in fp16 (1 cycle/row on the PE, fp32 PSUM accumulation).
Scores are computed transposed [keys, queries] so softmax needs no
transposes:
  - no max subtraction (scores ~N(0,1) scaled, exp stays in f16 range)
  - row sums via a ones-column appended to V (free in the ctx matmul)
  - causality: moving operand starts at the diagonal; one triangular mask
    multiply per diagonal chunk (on GPSIMD, off the critical engines)
  - 1/rowsum via DVE reciprocal_approx_fast on the PE-broadcast row sums
    (keeps ACT exp-only: no Ln/Exp table swaps)
Input x loaded with one big DMA per 1024-token block; q/k/v projection
phases ordered so PSUM->SBUF copies overlap the next phase's matmuls.
Output partials are written f16 (halves output DMA traffic).
"""
import sys

for _p in ("/opt/trn_rl_repo",):
    if _p not in sys.path:
        sys.path.insert(0, _p)

import numpy as np

import concourse.bass as bass
import concourse.mybir as mybir
import concourse.tile as tile
from concourse import bacc
from concourse.bass_utils import run_bass_kernel_spmd

B, S, D = 2, 2048, 1024
H, HD = 16, 64
T = B * S                      # 4096 tokens
NCORES = 8
HPC = H // NCORES              # heads per core = 2
CF = HPC * HD                  # per-core ctx features = 128
QBLK = 1024                    # query block width
NQB = S // QBLK                # 2 query blocks per batch
KCH = 128                      # key chunk
NFC = D // 128                 # contraction chunks for the projections
F16 = mybir.dt.float16
F32 = mybir.dt.float32
F32R = mybir.dt.float32r
AF = mybir.ActivationFunctionType
MUL = mybir.AluOpType.mult


def _emit_ctx_range(nc, cps, vp, probs, kc, nk, lo0, hi0):
    """ctx += V'.T @ probs over query columns [lo0, hi0), split at PSUM banks."""
    for s0 in range(0, QBLK, 512):
        lo = max(lo0, s0)
        hi = min(hi0, s0 + 512)
        if lo >= hi:
            continue
        nc.tensor.matmul(
            cps[:, lo:hi], vp[:, kc, :], probs[:, lo:hi],
            start=(kc == 0), stop=(kc == nk - 1),
        )


def _emit_outproj_tch(nc, ps_big, out_pool, part, wo_sb, ctx_sb, toff, q0, tch,
                      copy_eng="v"):
    """One 128-row slab of the deferred output projection. The PSUM->SBUF
    cast alternates between DVE and ACT so neither engine's backlog holds
    the PSUM rotation hostage."""
    ops = ps_big.tile([128, D], F32, tag="big")
    for s0 in range(0, D, 512):
        nc.tensor.matmul(
            ops[:, s0:s0 + 512],
            ctx_sb[:, tch * 128:(tch + 1) * 128],
            wo_sb[:, s0:s0 + 512],
            start=True, stop=True,
        )
    osb = out_pool.tile([128, D], F16, tag="o")
    if copy_eng == "s":
        nc.scalar.copy(osb[:, :], ops[:, :])
    else:
        nc.vector.tensor_copy(osb[:, :], ops[:, :])
    t0 = toff + q0 + tch * 128
    if tch % 2 == 0:
        nc.sync.dma_start(part[t0:t0 + 128, :], osb[:, :])
    else:
        nc.scalar.dma_start(part[t0:t0 + 128, :], osb[:, :])


def build_kernel():
    nc = bacc.Bacc()
    NTB = T // 1024
    # x and the qkv weights arrive pre-rearranged to partition-major layouts
    # so every load is one contiguous DMA per partition (no gather descriptors)
    xT = nc.dram_tensor("xT", [128, NTB, NFC, 1024], F16, kind="ExternalInput")
    wq = nc.dram_tensor("wq", [128, NFC, 128], F16, kind="ExternalInput")
    wk = nc.dram_tensor("wk", [128, NFC, 128], F16, kind="ExternalInput")
    wv = nc.dram_tensor("wv", [128, NFC, 128], F16, kind="ExternalInput")
    wo = nc.dram_tensor("wo", [CF, D], F16, kind="ExternalInput")
    tri = nc.dram_tensor("tri", [128, 128], F16, kind="ExternalInput")
    ide = nc.dram_tensor("ide", [128, 64], F16, kind="ExternalInput")
    ind2 = nc.dram_tensor("ind2", [2, 128], F32R, kind="ExternalInput")
    part = nc.dram_tensor("part", [T, D], F16, kind="ExternalOutput")

    with tile.TileContext(nc) as tc:
        with (
            tc.tile_pool(name="persist", bufs=1) as persist,
            tc.tile_pool(name="qkv_sb", bufs=1) as qkv_sb,
        ):
            # ---- weights / constants ----
            wq_sb = persist.tile([128, NFC, 128], F16, tag="wq")
            wk_sb = persist.tile([128, NFC, 128], F16, tag="wk")
            wv_sb = persist.tile([128, NFC, 128], F16, tag="wv")
            wo_sb = persist.tile([128, D], F16, tag="wo")
            tri_sb = persist.tile([128, 128], F16, tag="tri")
            ide_sb = persist.tile([128, 64], F16, tag="ide")
            indA_sb = persist.tile([1, 128], F32R, tag="indA")
            indB_sb = persist.tile([1, 128], F32R, tag="indB")
            # weight loads go on the scalar queue so the sync queue can lead
            # with the (much larger) first x-block DMA
            nc.scalar.dma_start(wq_sb[:, :, :], wq[:, :, :])
            nc.scalar.dma_start(wk_sb[:, :, :], wk[:, :, :])
            nc.scalar.dma_start(wv_sb[:, :, :], wv[:, :, :])
            ind_sbs = [indA_sb, indB_sb]

            # ---- persistent activations ----
            qt_sb = qkv_sb.tile([128, T], F16, tag="qt")    # Q_T [2*hd, T]
            kt_sb = qkv_sb.tile([128, T], F16, tag="kt")    # K_T
            vt_sb = qkv_sb.tile([128, T], F16, tag="vt")    # V_T

            # ================= Phase 1: QKV projections =================
            NTB = T // 1024
            with (
                tc.tile_pool(name="xp", bufs=NTB) as xp,
                tc.tile_pool(name="vp", bufs=4) as vp_pool,
                tc.tile_pool(name="probs", bufs=8) as probs_pool,
                tc.tile_pool(name="normp", bufs=4) as norm_pool,
                tc.tile_pool(name="recp", bufs=2) as rec_pool,
                tc.tile_pool(name="outp", bufs=3) as out_pool,
            ):

                def emit_vprep(b):
                    """V natural layout per head: [k-chunk 128, hd | 1].
                    Ones column via GPSIMD memset; features via DMA XBAR
                    transpose into a dense staging tile (the XBAR write
                    ignores output strides), then one strided DVE copy."""
                    toff = b * S
                    vps = []
                    for h in range(HPC):
                        hp = slice(h * HD, (h + 1) * HD)
                        vp = vp_pool.tile([128, S // KCH, HD + 1], F16, tag="vp")
                        vstage = vp_pool.tile([128, S // KCH, HD], F16, tag="vs")
                        nc.gpsimd.memset(vp[:, :, :], 1.0)
                        nc.sync.dma_start_transpose(
                            vstage[:, :, :], vt_sb[hp, toff:toff + S]
                        )
                        if b == 0:
                            # ACT is idle during phase 1; keep DVE clear of
                            # the projection copies
                            nc.scalar.copy(vp[:, :, 0:HD], vstage[:, :, :])
                        else:
                            nc.vector.tensor_copy(vp[:, :, 0:HD], vstage[:, :, :])
                        vps.append(vp)
                    return vps

                with (
                    tc.tile_pool(name="qkv_ps", bufs=1, space="PSUM") as qkv_ps,
                ):
                    xts = []
                    for tb in range(NTB):
                        xt = xp.tile([128, NFC, 1024], F16, tag="x")
                        if tb == 0:
                            # split the first block across both HWDGE queues so
                            # their DGE warmups overlap and compute starts sooner
                            nc.sync.dma_start(xt[:, 0:4, :], xT[:, 0, 0:4, :])
                            nc.scalar.dma_start(xt[:, 4:8, :], xT[:, 0, 4:8, :])
                        else:
                            nc.sync.dma_start(xt[:, :, :], xT[:, tb, :, :])
                        xts.append(xt)
                    # phase-2 constants: queued after the x blocks so they
                    # don't delay the first projections through DGE warmup
                    nc.scalar.dma_start(wo_sb[:, :], wo[:, :])
                    nc.scalar.dma_start(tri_sb[:, :], tri[:, :])
                    nc.scalar.dma_start(ide_sb[:, :], ide[:, :])
                    nc.scalar.dma_start(indA_sb[:, :], ind2[0:1, :])
                    nc.scalar.dma_start(indB_sb[:, :], ind2[1:2, :])
                    vps0 = None
                    for tb in range(NTB):
                        if tb == 2:
                            # batch 0 V prep as soon as its tokens are done:
                            # runs on GPSIMD/DMA/DVE behind the remaining
                            # projection matmuls
                            vps0 = emit_vprep(0)
                        xt = xts[tb]
                        sl = slice(tb * 1024, (tb + 1) * 1024)
                        # q -> k -> v phase order: each phase's PSUM->SBUF copy
                        # overlaps the next phase's matmuls (separate buffers).
                        for w_sb, dst, tg in (
                            (wq_sb, qt_sb, "pq"), (wk_sb, kt_sb, "pk"), (wv_sb, vt_sb, "pv")
                        ):
                            ps = qkv_ps.tile([128, 1024], F32, tag=tg)
                            for s0 in range(0, 1024, 512):
                                for f in range(NFC):
                                    nc.tensor.matmul(
                                        ps[:, s0:s0 + 512], w_sb[:, f, :],
                                        xt[:, f, s0:s0 + 512],
                                        start=(f == 0), stop=(f == NFC - 1),
                                    )
                            nc.vector.tensor_copy(dst[:, sl], ps[:, :])

                # ============== Phase 2: attention + out-proj ==============
                with (
                    tc.tile_pool(name="ps_big", bufs=2, space="PSUM") as ps_big,
                    tc.tile_pool(name="ps_ctx", bufs=2, space="PSUM") as ps_ctx,
                ):
                    pending_out = None   # (ctx_sb, toff, q0) of the previous block
                    pending_norm = None  # (rws, ctx_sb): deferred recb mm + mult

                    def emit_norm_tail(pn):
                        rws_, csb_ = pn
                        recb_ps = ps_big.tile([128, QBLK], F32, tag="big")
                        for s0 in range(0, QBLK, 512):
                            for h in range(HPC):
                                nc.tensor.matmul(
                                    recb_ps[:, s0:s0 + 512],
                                    ind_sbs[h][:, :], rws_[h][:, s0:s0 + 512],
                                    start=(h == 0), stop=(h == HPC - 1),
                                )
                        recf = rec_pool.tile([128, QBLK], F32, tag="recf")
                        nc.vector.reciprocal_approx_fast(recf[:, :], recb_ps[:, :])
                        nc.vector.tensor_tensor(
                            csb_[:, :], csb_[:, :], recf[:, :], MUL,
                        )

                    vps_next = vps0  # emitted mid-phase-1
                    for b in range(B):
                        toff = b * S
                        vps = vps_next

                        qb_order = range(NQB) if b < B - 1 else (1, 0)
                        for qb in qb_order:
                            q0 = qb * QBLK
                            nk = (q0 + QBLK) // KCH
                            if qb == NQB - 1 and b + 1 < B:  # b<B-1: in-order
                                # next batch's V prep early: runs on GPSIMD/DMA
                                # while this block's matmuls proceed
                                vps_next = emit_vprep(b + 1)
                            ctx_sb = norm_pool.tile([128, QBLK], F16, tag="ctx")
                            cps_list = []
                            for _h in range(HPC):
                                cps_t = ps_ctx.tile([HD + 1, QBLK], F32, tag="cps")
                                cps_list.append(cps_t)
                            out_spread = 0
                            # Both heads' chunk pipelines interleaved: two
                            # independent streams double the work per sync hop so
                            # neither PE nor ACT idles on semaphore latency.
                            pa = [None] * HPC  # (probs, kc, off) -> pending ctx
                            for kc in range(nk):
                                off = max(0, kc * KCH - q0)
                                diag = kc * KCH >= q0
                                pr = []
                                for h in range(HPC):
                                    hp = slice(h * HD, (h + 1) * HD)
                                    sps = ps_big.tile([128, QBLK], F32, tag="big")
                                    probs = probs_pool.tile([128, QBLK], F16, tag="p")
                                    pr.append(probs)
                                    for s0 in range(0, QBLK, 512):
                                        lo = max(off, s0)
                                        hi = s0 + 512
                                        if lo >= hi:
                                            continue
                                        nc.tensor.matmul(
                                            sps[:, lo:hi],
                                            kt_sb[hp, toff + kc * KCH: toff + (kc + 1) * KCH],
                                            qt_sb[hp, toff + q0 + lo: toff + q0 + hi],
                                            start=True, stop=True,
                                        )
                                    nc.scalar.activation(
                                        probs[:, off:], sps[:, off:], AF.Exp,
                                        bias=0.0, scale=0.125,
                                    )
                                    if diag:
                                        # triangular mask on the straddle (GPSIMD)
                                        nc.gpsimd.tensor_tensor(
                                            probs[:, off:off + KCH],
                                            probs[:, off:off + KCH],
                                            tri_sb[:, :],
                                            MUL,
                                        )
                                for h in range(HPC):
                                    if pa[h] is not None:
                                        p_, k_, o_ = pa[h]
                                        _emit_ctx_range(
                                            nc, cps_list[h], vps[h], p_, k_, nk, o_, QBLK
                                        )
                                    pa[h] = (pr[h], kc, off)
                                # deferred normalize tail, then spread the previous
                                # block's out-projection as PE filler
                                if kc >= 1 and pending_norm is not None:
                                    emit_norm_tail(pending_norm)
                                    pending_norm = None
                                elif kc >= 1 and pending_out is not None and out_spread < QBLK // 128:
                                    final_blk = b == B - 1 and qb == 0
                                    if not final_blk or out_spread < (kc * (QBLK // 128)) // nk:
                                        # final block: pace the slabs across all
                                        # chunks — its late chunks have no other
                                        # PE filler
                                        _emit_outproj_tch(
                                            nc, ps_big, out_pool, part, wo_sb,
                                            pending_out[0], pending_out[1],
                                            pending_out[2], out_spread,
                                            copy_eng="sv"[out_spread % 2],
                                        )
                                        out_spread += 1
                            # drain the pipelines
                            for h in range(HPC):
                                if pa[h] is not None:
                                    p_, k_, o_ = pa[h]
                                    _emit_ctx_range(
                                        nc, cps_list[h], vps[h], p_, k_, nk, o_, QBLK
                                    )
                            # flush any out-projection slabs that didn't fit
                            while pending_out is not None and out_spread < QBLK // 128:
                                _emit_outproj_tch(
                                    nc, ps_big, out_pool, part, wo_sb,
                                    pending_out[0], pending_out[1],
                                    pending_out[2], out_spread,
                                    copy_eng="sv"[out_spread % 2],
                                )
                                out_spread += 1
                            # extract rowsums + unnormalized ctx (DVE), freeing
                            # the PSUM accumulators for the next block.
                            rws = []
                            for h in range(HPC):
                                rrow = norm_pool.tile([1, QBLK], F32R, tag="rrow")
                                for s0 in range(0, QBLK, 512):
                                    nc.vector.tensor_copy(
                                        rrow[:, s0:s0 + 512],
                                        cps_list[h][HD:HD + 1, s0:s0 + 512],
                                    )
                                rws.append(rrow)
                            last_block = b == B - 1 and qb == 0
                            for h in range(HPC):
                                hp = slice(h * HD, (h + 1) * HD)
                                if last_block:
                                    # ACT is idle at the tail; keep DVE free
                                    # for the reciprocal chain
                                    nc.scalar.copy(
                                        ctx_sb[hp, :], cps_list[h][0:HD, :]
                                    )
                                else:
                                    nc.vector.tensor_copy(
                                        ctx_sb[hp, :], cps_list[h][0:HD, :]
                                    )
                            pending_norm = (rws, ctx_sb)
                            pending_out = (ctx_sb, toff, q0)
                    # trailing normalize + out-projection for the final block,
                    # pipelined at 512-column granularity so the out-proj matmuls
                    # of the first half overlap the second half's normalization.
                    rws_, csb_ = pending_norm
                    recf = rec_pool.tile([128, QBLK], F32, tag="recf")
                    for s0 in range(0, QBLK, 512):
                        recb_ps = ps_big.tile([128, 512], F32, tag="big")
                        for h in range(HPC):
                            nc.tensor.matmul(
                                recb_ps[:, :],
                                ind_sbs[h][:, :], rws_[h][:, s0:s0 + 512],
                                start=(h == 0), stop=(h == HPC - 1),
                            )
                        nc.vector.reciprocal_approx_fast(
                            recf[:, s0:s0 + 512], recb_ps[:, :]
                        )
                        nc.vector.tensor_tensor(
                            csb_[:, s0:s0 + 512], csb_[:, s0:s0 + 512],
                            recf[:, s0:s0 + 512], MUL,
                        )
                    # slabs after the full norm chain: first slabs start when
                    # the first segment's multiply lands; DVE stays on recip
                    for tch in range(QBLK // 128):
                        _emit_outproj_tch(
                            nc, ps_big, out_pool, part, wo_sb,
                            pending_out[0], pending_out[1], pending_out[2], tch,
                            copy_eng="sv"[tch % 2],
                        )
    nc.compile()
    return nc


_NC_CACHE = None


def _get_nc():
    global _NC_CACHE
    if _NC_CACHE is None:
        _NC_CACHE = build_kernel()
    return _NC_CACHE


def _warr(w):
    """[D, CF] torch-style slice -> partition-major [128, NFC, 128]."""
    return np.ascontiguousarray(
        w.astype(np.float16).reshape(NFC, 128, CF).transpose(1, 0, 2)
    )


def make_in_maps(x, Wq, Wk, Wv, Wo):
    NTB = T // 1024
    xf = x.reshape(T, D).astype(np.float16)
    # partition-major blocks: xT[p, tb, c, u] = x[tb*1024+u, c*128+p]
    xT = np.ascontiguousarray(
        xf.reshape(NTB, 1024, NFC, 128).transpose(3, 0, 2, 1)
    )
    tri = np.triu(np.ones((128, 128), dtype=np.float16))
    ide = np.concatenate([np.eye(64, dtype=np.float16)] * 2, axis=0)
    ind2 = np.zeros((2, 128), dtype=np.float32)
    ind2[0, 0:64] = 1.0
    ind2[1, 64:128] = 1.0
    in_maps = []
    for c in range(NCORES):
        rs = slice(c * CF, (c + 1) * CF)
        in_maps.append({
            "xT": xT,
            "wq": _warr(Wq[rs, :].T),
            "wk": _warr(Wk[rs, :].T),
            "wv": _warr(Wv[rs, :].T),
            "wo": np.ascontiguousarray(Wo[:, rs].T.astype(np.float16)),
            "tri": tri,
            "ide": ide,
            "ind2": ind2,
        })
    return in_maps


def kernel(x, Wq, Wk, Wv, Wo, bo):
    x = np.asarray(x, dtype=np.float32)
    Wq = np.asarray(Wq, dtype=np.float32)
    Wk = np.asarray(Wk, dtype=np.float32)
    Wv = np.asarray(Wv, dtype=np.float32)
    Wo = np.asarray(Wo, dtype=np.float32)
    bo = np.asarray(bo, dtype=np.float32)

    in_maps = make_in_maps(x, Wq, Wk, Wv, Wo)
    res = run_bass_kernel_spmd(_get_nc(), in_maps, core_ids=list(range(NCORES)))
    out = res.results[0]["part"].astype(np.float32)
    for c in range(1, NCORES):
        out += res.results[c]["part"].astype(np.float32)
    out += bo[None, :]
    return out.reshape(B, S, D)



# revision 4
# speedup vs baseline: 1.0647x; 1.0647x over previous
"""Multi-head causal attention (B=2, S=2048, D=1024, H=16, hd=64) on 8 TRN2
NeuronCores.

Sharding: tensor-parallel over heads - 2 heads per core. Each core computes
Q/K/V for its 2 heads over the full sequence, causal attention, and a partial
output projection (its 128 context features x Wo slice). Host sums the 8
partials (f16) and adds the bias.

v2 design:
  - Scores matmuls (K=hd=64) run as ROW-TILED HEAD PAIRS: h0 on PE rows 0-63,
    h1 on rows 64-127 (auto tile_position from base partitions), concurrent.
    Halves score PE time vs serial heads.
  - One exp (ACT) per key chunk covering both heads ([128, 2, N] psum tile):
    halves ACT instruction count.
  - ctx matmuls stay M=65 (V + ones column for free row sums), serial heads.
  - QKV projection is interleaved block-by-block with attention so the
    ACT-bound attention inner loop overlaps the PE-bound projection matmuls.
  - Deferred per-block normalize + out-projection slabs fill remaining PE
    gaps (as in v1), with the reciprocal broadcast done by K=1 matmuls.
  - QBLK=512 so all PSUM pools (scores 2x2 banks, ctx 2x1, mm 2x1) fit the
    8-bank budget exactly.
"""
import sys
from collections import deque

for _p in ("/opt/trn_rl_repo",):
    if _p not in sys.path:
        sys.path.insert(0, _p)

import numpy as np

import concourse.bass as bass
import concourse.mybir as mybir
import concourse.tile as tile
from concourse import bacc
from concourse.bass_utils import run_bass_kernel_spmd

B, S, D = 2, 2048, 1024
H, HD = 16, 64
T = B * S                      # 4096 tokens
NCORES = 8
HPC = H // NCORES              # heads per core = 2
CF = HPC * HD                  # per-core ctx features = 128
QBLK = 512                     # query block width
NQB = S // QBLK                # 4 query blocks per batch
KCH = 128                      # key chunk
NFC = D // 128                 # contraction chunks for the projections
NTB = T // 1024                # 1024-token x blocks
F16 = mybir.dt.float16
F32 = mybir.dt.float32
F32R = mybir.dt.float32r
AF = mybir.ActivationFunctionType
MUL = mybir.AluOpType.mult


def build_kernel():
    nc = bacc.Bacc()
    # x and the qkv weights arrive pre-rearranged to partition-major layouts
    # so every load is one contiguous DMA per partition
    xT = nc.dram_tensor("xT", [128, NTB, NFC, 1024], F16, kind="ExternalInput")
    wq = nc.dram_tensor("wq", [128, NFC, 128], F16, kind="ExternalInput")
    wk = nc.dram_tensor("wk", [128, NFC, 128], F16, kind="ExternalInput")
    wv = nc.dram_tensor("wv", [128, NFC, 128], F16, kind="ExternalInput")
    wo = nc.dram_tensor("wo", [CF, D], F16, kind="ExternalInput")
    tri = nc.dram_tensor("tri", [128, 2, 128], F16, kind="ExternalInput")
    ind2 = nc.dram_tensor("ind2", [2, 128], F32R, kind="ExternalInput")
    part = nc.dram_tensor("part", [T, D], F16, kind="ExternalOutput")

    with tile.TileContext(nc) as tc:
        with (
            tc.tile_pool(name="persist", bufs=1) as persist,
            tc.tile_pool(name="qkv_sb", bufs=1) as qkv_sb,
        ):
            # ---- weights / constants ----
            wq_sb = persist.tile([128, NFC, 128], F16, tag="wq")
            wk_sb = persist.tile([128, NFC, 128], F16, tag="wk")
            wv_sb = persist.tile([128, NFC, 128], F16, tag="wv")
            wo_sb = persist.tile([128, D], F16, tag="wo")
            tri_sb = persist.tile([128, 2, 128], F16, tag="tri")
            indA_sb = persist.tile([1, 128], F32R, tag="indA")
            indB_sb = persist.tile([1, 128], F32R, tag="indB")
            warm_in = persist.tile([1, 16], F32, tag="wi")
            warm_out = persist.tile([1, 16], F16, tag="wo2")
            ind_sbs = [indA_sb, indB_sb]

            # exp table load happens during the projection phase, not at the
            # first attention chunk
            nc.gpsimd.memset(warm_in[:, :], 0.0)
            nc.scalar.activation(warm_out[:, :], warm_in[:, :], AF.Exp,
                                 bias=0.0, scale=0.125)

            # weight loads on the scalar queue so the sync queue leads with
            # the (much larger) first x-block DMA
            nc.scalar.dma_start(wq_sb[:, :, :], wq[:, :, :])
            nc.scalar.dma_start(wk_sb[:, :, :], wk[:, :, :])
            nc.scalar.dma_start(wv_sb[:, :, :], wv[:, :, :])

            # ---- persistent activations ----
            qt_sb = qkv_sb.tile([128, T], F16, tag="qt")    # Q_T [2*hd, T]
            kt_sb = qkv_sb.tile([128, T], F16, tag="kt")    # K_T
            vt_sb = qkv_sb.tile([128, T], F16, tag="vt")    # V_T

            with (
                tc.tile_pool(name="xp", bufs=NTB) as xp,
                tc.tile_pool(name="sps_ps", bufs=2, space="PSUM") as sps_ps,
                tc.tile_pool(name="ctx_ps", bufs=1, space="PSUM") as ctx_ps,
                tc.tile_pool(name="mm_ps", bufs=2, space="PSUM") as mm_ps,
                tc.tile_pool(name="probs", bufs=4) as probs_pool,
                tc.tile_pool(name="vstage", bufs=2) as vstage_pool,
                tc.tile_pool(name="vpp", bufs=6) as vp_pool,
                tc.tile_pool(name="normp", bufs=3) as norm_pool,
                tc.tile_pool(name="rrp", bufs=4) as rr_pool,
                tc.tile_pool(name="recp", bufs=2) as rec_pool,
                tc.tile_pool(name="outp", bufs=4) as out_pool,
            ):
                # ---- x DMAs, all queued up front ----
                xts = []
                for tb in range(NTB):
                    xt = xp.tile([128, NFC, 1024], F16, tag="x")
                    if tb == 0:
                        # split the first block across both HWDGE queues so
                        # their DGE warmups overlap and compute starts sooner
                        nc.sync.dma_start(xt[:, 0:4, :], xT[:, 0, 0:4, :])
                        nc.scalar.dma_start(xt[:, 4:8, :], xT[:, 0, 4:8, :])
                    else:
                        nc.sync.dma_start(xt[:, :, :], xT[:, tb, :, :])
                    xts.append(xt)
                # phase-2 constants: queued after the x blocks so they don't
                # delay the first projections through DGE warmup
                nc.scalar.dma_start(wo_sb[:, :], wo[:, :])
                nc.scalar.dma_start(tri_sb[:, :, :], tri[:, :, :])
                nc.scalar.dma_start(indA_sb[:, :], ind2[0:1, :])
                nc.scalar.dma_start(indB_sb[:, :], ind2[1:2, :])

                vp_map = {}

                def qkv_group(tb, w_sb, dst, s0):
                    def emit():
                        ps = mm_ps.tile([128, 512], F32, tag="mm")
                        xt = xts[tb]
                        for f in range(NFC):
                            nc.tensor.matmul(
                                ps[:, :], w_sb[:, f, :], xt[:, f, s0:s0 + 512],
                                start=(f == 0), stop=(f == NFC - 1),
                            )
                        nc.vector.tensor_copy(
                            dst[:, tb * 1024 + s0: tb * 1024 + s0 + 512],
                            ps[:, :],
                        )
                    return emit

                def qkv_groups(tb):
                    gs = []
                    for w_sb, dst in ((wq_sb, qt_sb), (wk_sb, kt_sb),
                                      (wv_sb, vt_sb)):
                        for s0 in (0, 512):
                            gs.append(qkv_group(tb, w_sb, dst, s0))
                    return gs

                def vprep(tb):
                    def emit():
                        for h in range(HPC):
                            hp = slice(h * HD, (h + 1) * HD)
                            vstage = vstage_pool.tile([128, 8, HD], F16, tag="vs")
                            nc.sync.dma_start_transpose(
                                vstage[:, :, :],
                                vt_sb[hp, tb * 1024:(tb + 1) * 1024],
                            )
                            vp = vp_pool.tile([128, 8, HD + 1], F16, tag="vp")
                            nc.gpsimd.memset(vp[:, :, HD:HD + 1], 1.0)
                            nc.gpsimd.tensor_copy(vp[:, :, 0:HD], vstage[:, :, :])
                            vp_map[(tb, h)] = vp
                    return emit

                # state threaded between attention blocks
                pend_norm = [None]   # (cps pair, cell for ctx_sb result)
                pend_out = [None]    # (cell, toff, q0)
                dma_alt = [0]

                def emit_norm():
                    cps, cell = pend_norm[0]
                    pend_norm[0] = None
                    rr = []
                    for h in range(HPC):
                        r = rr_pool.tile([1, QBLK], F32R, tag="rr")
                        nc.vector.tensor_copy(r[:, :], cps[h][HD:HD + 1, :])
                        rr.append(r)
                    recb = mm_ps.tile([128, QBLK], F32, tag="mm")
                    for h in range(HPC):
                        nc.tensor.matmul(
                            recb[:, :], ind_sbs[h][:, :], rr[h][:, :],
                            start=(h == 0), stop=(h == HPC - 1),
                        )
                    recf = rec_pool.tile([128, QBLK], F32, tag="recf")
                    nc.vector.reciprocal_approx_fast(recf[:, :], recb[:, :])
                    ctx_sb = norm_pool.tile([128, QBLK], F16, tag="ctx")
                    # fused psum-read + normalize + f16 cast
                    nc.vector.tensor_tensor(
                        ctx_sb[0:HD, :], cps[0][0:HD, :], recf[0:HD, :], MUL)
                    nc.vector.tensor_tensor(
                        ctx_sb[HD:128, :], cps[1][0:HD, :], recf[HD:128, :], MUL)
                    cell[0] = ctx_sb

                def outproj_half(tch, s0):
                    def emit():
                        cell, toff, q0 = pend_out[0]
                        ctx_sb = cell[0]
                        ops = mm_ps.tile([128, 512], F32, tag="mm")
                        nc.tensor.matmul(
                            ops[:, :],
                            ctx_sb[:, tch * 128:(tch + 1) * 128],
                            wo_sb[:, s0:s0 + 512],
                            start=True, stop=True,
                        )
                        osb = out_pool.tile([128, 512], F16, tag="o")
                        nc.vector.tensor_copy(osb[:, :], ops[:, :])
                        t0 = toff + q0 + tch * 128
                        if dma_alt[0] % 2 == 0:
                            nc.sync.dma_start(part[t0:t0 + 128, s0:s0 + 512],
                                              osb[:, :])
                        else:
                            nc.gpsimd.dma_start(part[t0:t0 + 128, s0:s0 + 512],
                                                osb[:, :])
                        dma_alt[0] += 1
                    return emit

                def emit_ctx(b, cps, pend_chunk, nk):
                    probs, c, off = pend_chunk
                    tbv = b * 2 + (c * 128) // 1024
                    for h in range(HPC):
                        src = vp_map[(tbv, h)][:, (c * 128 % 1024) // 128, :]
                        nc.tensor.matmul(
                            cps[h][:, off:QBLK], src, probs[:, h, off:QBLK],
                            start=(c == 0), stop=(c == nk - 1),
                        )

                def att_block(b, qb, fillers):
                    toff, q0 = b * S, qb * QBLK
                    nk = (q0 + QBLK) // KCH
                    cps0 = ctx_ps.tile([HD + 1, QBLK], F32, tag="c0")
                    cps1 = ctx_ps.tile([HD + 1, QBLK], F32, tag="c1")
                    cps = [cps0, cps1]
                    pend_chunk = None
                    fq = deque(fillers)
                    for c in range(nk):
                        off = max(0, 128 * c - q0)
                        sps = sps_ps.tile([128, 2, QBLK], F32, tag="sps")
                        for h in range(HPC):
                            hp = slice(h * HD, (h + 1) * HD)
                            nc.tensor.matmul(
                                sps[:, h, off:QBLK],
                                kt_sb[hp, toff + c * 128: toff + (c + 1) * 128],
                                qt_sb[hp, toff + q0 + off: toff + q0 + QBLK],
                                start=True, stop=True,
                            )
                        probs = probs_pool.tile([128, 2, QBLK], F16, tag="p")
                        nc.scalar.activation(
                            probs[:, :, off:], sps[:, :, off:], AF.Exp,
                            bias=0.0, scale=0.125,
                        )
                        if c * 128 >= q0:
                            # triangular mask on the diagonal straddle, both
                            # heads in one GPSIMD op
                            nc.gpsimd.tensor_tensor(
                                probs[:, :, off:off + 128],
                                probs[:, :, off:off + 128],
                                tri_sb[:, :, :], MUL,
                            )
                        if c == 0:
                            # previous block's normalize first: frees its ctx
                            # psum accumulators before this block's first ctx
                            if pend_norm[0] is not None:
                                emit_norm()
                        else:
                            if pend_chunk is not None:
                                emit_ctx(b, cps, pend_chunk, nk)
                            # pace remaining fillers over remaining chunks
                            rem = nk - c
                            want = (len(fq) + rem - 1) // rem
                            for _ in range(min(want, len(fq))):
                                fq.popleft()()
                        pend_chunk = (probs, c, off)
                    emit_ctx(b, cps, pend_chunk, nk)
                    while fq:
                        fq.popleft()()
                    cell = [None]
                    pend_norm[0] = (cps, cell)
                    pend_out[0] = (cell, toff, q0)

                # ================= emission =================
                for g in qkv_groups(0):
                    g()
                vprep(0)()

                for tb in range(NTB):
                    b = tb // 2
                    qbA = 2 * (tb % 2)
                    qbB = qbA + 1
                    nkA = (qbA + 1) * (QBLK // KCH)
                    nkB = (qbB + 1) * (QBLK // KCH)
                    if tb < NTB - 1:
                        nxt = qkv_groups(tb + 1)
                        nsplit = max(1, round(len(nxt) * nkA / (nkA + nkB)))
                        fillA = nxt[:nsplit]
                        fillB = nxt[nsplit:] + [vprep(tb + 1)]
                    else:
                        fillA, fillB = [], []
                    outs = ([outproj_half(tch, s0) for tch in range(QBLK // 128)
                             for s0 in (0, 512)] if pend_out[0] else [])
                    att_block(b, qbA, outs + fillA)
                    outs = [outproj_half(tch, s0) for tch in range(QBLK // 128)
                            for s0 in (0, 512)]
                    att_block(b, qbB, outs + fillB)

                # trailing normalize + out-projection for the final block
                emit_norm()
                for tch in range(QBLK // 128):
                    for s0 in (0, 512):
                        outproj_half(tch, s0)()
    nc.compile()
    return nc


_NC_CACHE = None


def _get_nc():
    global _NC_CACHE
    if _NC_CACHE is None:
        _NC_CACHE = build_kernel()
    return _NC_CACHE


def _warr(w):
    """[D, CF] torch-style slice -> partition-major [128, NFC, 128]."""
    return np.ascontiguousarray(
        w.astype(np.float16).reshape(NFC, 128, CF).transpose(1, 0, 2)
    )


def make_in_maps(x, Wq, Wk, Wv, Wo):
    xf = x.reshape(T, D).astype(np.float16)
    # partition-major blocks: xT[p, tb, c, u] = x[tb*1024+u, c*128+p]
    xT = np.ascontiguousarray(
        xf.reshape(NTB, 1024, NFC, 128).transpose(3, 0, 2, 1)
    )
    tri1 = np.triu(np.ones((128, 128), dtype=np.float16))
    tri = np.ascontiguousarray(np.stack([tri1, tri1], axis=1))
    ind2 = np.zeros((2, 128), dtype=np.float32)
    ind2[0, 0:64] = 1.0
    ind2[1, 64:128] = 1.0
    in_maps = []
    for c in range(NCORES):
        rs = slice(c * CF, (c + 1) * CF)
        in_maps.append({
            "xT": xT,
            "wq": _warr(Wq[rs, :].T),
            "wk": _warr(Wk[rs, :].T),
            "wv": _warr(Wv[rs, :].T),
            "wo": np.ascontiguousarray(Wo[:, rs].T.astype(np.float16)),
            "tri": tri,
            "ind2": ind2,
        })
    return in_maps


def kernel(x, Wq, Wk, Wv, Wo, bo):
    x = np.asarray(x, dtype=np.float32)
    Wq = np.asarray(Wq, dtype=np.float32)
    Wk = np.asarray(Wk, dtype=np.float32)
    Wv = np.asarray(Wv, dtype=np.float32)
    Wo = np.asarray(Wo, dtype=np.float32)
    bo = np.asarray(bo, dtype=np.float32)

    in_maps = make_in_maps(x, Wq, Wk, Wv, Wo)
    res = run_bass_kernel_spmd(_get_nc(), in_maps, core_ids=list(range(NCORES)))
    out = res.results[0]["part"].astype(np.float32)
    for c in range(1, NCORES):
        out += res.results[c]["part"].astype(np.float32)
    out += bo[None, :]
    return out.reshape(B, S, D)


# revision 14
# speedup vs baseline: 1.0842x; 1.0183x over previous
"""Multi-head causal attention (B=2, S=2048, D=1024, H=16, hd=64) on 8 TRN2
NeuronCores.

Sharding: tensor-parallel over heads - 2 heads per core. Each core computes
Q/K/V for its 2 heads over the full sequence, causal attention, and a partial
output projection (its 128 context features x Wo slice). Host sums the 8
partials (f16) and adds the bias.

v4 design (all f16 matmuls, fp32 PSUM):
  - Scores matmuls (K=hd=64) run as row-tiled head pairs: h0 on PE rows
    0-63, h1 on rows 64-127 (concurrent via auto tile_position from the
    operands' base partitions). ~1.35x over serial heads (bus arbitration
    limits the ideal 2x).
  - One exp (ACT) per key chunk covering both heads ([128, 2, N] psum
    tile spanning 2 banks): halves ACT instruction count vs per-head.
  - ctx matmuls M=65 (V + ones column for free row sums), serial heads;
    moving-operand bus is fully utilized so this is already at roofline.
  - Per-block softmax denominators: rowsum row extract + K=1 matmul
    broadcast + DVE reciprocal all in the block TAIL; only the two fused
    normalize-multiplies sit on the next block's critical path.
  - QKV projection interleaved chunk-by-chunk with attention (the
    ACT-bound attention loop overlaps the PE-bound projections), deferred
    out-projection slabs as additional PE filler.
  - Bulk x prefetch spread across the scalar and gpsimd DGE queues so the
    latency-critical sync-queue DMAs (first x block, V transposes, output
    slabs) are never stuck behind megabytes of prefetch.
"""
import sys
from collections import deque

for _p in ("/opt/trn_rl_repo",):
    if _p not in sys.path:
        sys.path.insert(0, _p)

import numpy as np

import concourse.bass as bass
import concourse.mybir as mybir
import concourse.tile as tile
from concourse import bacc
from concourse.bass_utils import run_bass_kernel_spmd

B, S, D = 2, 2048, 1024
H, HD = 16, 64
T = B * S                      # 4096 tokens
NCORES = 8
HPC = H // NCORES              # heads per core = 2
CF = HPC * HD                  # per-core ctx features = 128
QBLK = 512                     # query block width
NQB = S // QBLK                # 4 query blocks per batch
KCH = 128                      # key chunk
NFC = D // 128                 # contraction chunks for the projections
NTB = T // 1024                # 1024-token x blocks
F16 = mybir.dt.float16
F32 = mybir.dt.float32
F32R = mybir.dt.float32r
AF = mybir.ActivationFunctionType
MUL = mybir.AluOpType.mult


def build_kernel():
    nc = bacc.Bacc()
    xT = nc.dram_tensor("xT", [128, NTB, NFC, 1024], F16, kind="ExternalInput")
    wq = nc.dram_tensor("wq", [128, NFC, 128], F16, kind="ExternalInput")
    wk = nc.dram_tensor("wk", [128, NFC, 128], F16, kind="ExternalInput")
    wv = nc.dram_tensor("wv", [128, NFC, 128], F16, kind="ExternalInput")
    wo = nc.dram_tensor("wo", [CF, D], F16, kind="ExternalInput")
    tri = nc.dram_tensor("tri", [128, 2, 128], F16, kind="ExternalInput")
    ind2 = nc.dram_tensor("ind2", [2, 128], F32R, kind="ExternalInput")
    part = nc.dram_tensor("part", [T, D], F16, kind="ExternalOutput")

    with tile.TileContext(nc) as tc:
        with (
            tc.tile_pool(name="persist", bufs=1) as persist,
            tc.tile_pool(name="qkv_sb", bufs=1) as qkv_sb,
        ):
            # ---- weights / constants ----
            wq_sb = persist.tile([128, NFC, 128], F16, tag="wq")
            wk_sb = persist.tile([128, NFC, 128], F16, tag="wk")
            wv_sb = persist.tile([128, NFC, 128], F16, tag="wv")
            wo_sb = persist.tile([128, D], F16, tag="wo")
            tri_sb = persist.tile([128, 2, 128], F16, tag="tri")
            indA_sb = persist.tile([1, 128], F32R, tag="indA")
            indB_sb = persist.tile([1, 128], F32R, tag="indB")
            ind_sbs = [indA_sb, indB_sb]
            warm_in = persist.tile([1, 16], F32, tag="wi")
            warm_out = persist.tile([1, 16], F16, tag="wo2")

            # exp table load happens during the projection phase
            nc.gpsimd.memset(warm_in[:, :], 0.0)
            nc.scalar.activation(warm_out[:, :], warm_in[:, :], AF.Exp,
                                 bias=0.0, scale=0.125)

            nc.scalar.dma_start(wq_sb[:, :, :], wq[:, :, :])
            nc.scalar.dma_start(wk_sb[:, :, :], wk[:, :, :])
            nc.scalar.dma_start(wv_sb[:, :, :], wv[:, :, :])

            # ---- persistent activations ----
            qt_sb = qkv_sb.tile([128, T], F16, tag="qt")
            kt_sb = qkv_sb.tile([128, T], F16, tag="kt")
            vt_sb = qkv_sb.tile([128, T], F16, tag="vt")

            with (
                tc.tile_pool(name="xp", bufs=NTB) as xp,
                tc.tile_pool(name="sps_ps", bufs=2, space="PSUM") as sps_ps,
                tc.tile_pool(name="ctx_ps", bufs=1, space="PSUM") as ctx_ps,
                tc.tile_pool(name="mm_ps", bufs=2, space="PSUM") as mm_ps,
                tc.tile_pool(name="probs", bufs=4) as probs_pool,
                tc.tile_pool(name="vstage", bufs=2) as vstage_pool,
                tc.tile_pool(name="vpp", bufs=6) as vp_pool,
                tc.tile_pool(name="normp", bufs=3) as norm_pool,
                tc.tile_pool(name="rrp", bufs=4) as rr_pool,
                tc.tile_pool(name="recp", bufs=2) as rec_pool,
                tc.tile_pool(name="outp", bufs=4) as out_pool,
            ):
                # ---- x DMAs: tb0 split sync/scalar (needed first); the
                # bulk prefetch rides the scalar + gpsimd queues so it
                # never delays the sync queue's transposes/output slabs ----
                xts = []
                for tb in range(NTB):
                    xt = xp.tile([128, NFC, 1024], F16, tag="x")
                    if tb == 0:
                        nc.sync.dma_start(xt[:, 0:4, :], xT[:, 0, 0:4, :])
                        nc.scalar.dma_start(xt[:, 4:8, :], xT[:, 0, 4:8, :])
                    elif tb == 1:
                        nc.scalar.dma_start(xt[:, :, :], xT[:, tb, :, :])
                    else:
                        nc.gpsimd.dma_start(xt[:, :, :], xT[:, tb, :, :])
                    xts.append(xt)
                nc.scalar.dma_start(wo_sb[:, :], wo[:, :])
                nc.scalar.dma_start(tri_sb[:, :, :], tri[:, :, :])
                nc.scalar.dma_start(indA_sb[:, :], ind2[0:1, :])
                nc.scalar.dma_start(indB_sb[:, :], ind2[1:2, :])

                vp_map = {}

                def qkv_group(tb, w_sb, dst, s0):
                    def emit():
                        ps = mm_ps.tile([128, 512], F32, tag="mm")
                        xt = xts[tb]
                        for f in range(NFC):
                            nc.tensor.matmul(
                                ps[:, :], w_sb[:, f, :], xt[:, f, s0:s0 + 512],
                                start=(f == 0), stop=(f == NFC - 1),
                            )
                        nc.vector.tensor_copy(
                            dst[:, tb * 1024 + s0: tb * 1024 + s0 + 512],
                            ps[:, :],
                        )
                    return emit

                def qkv_groups(tb):
                    return [
                        qkv_group(tb, wq_sb, qt_sb, 0),
                        qkv_group(tb, wk_sb, kt_sb, 0),
                        qkv_group(tb, wq_sb, qt_sb, 512),
                        qkv_group(tb, wk_sb, kt_sb, 512),
                        qkv_group(tb, wv_sb, vt_sb, 0),
                        qkv_group(tb, wv_sb, vt_sb, 512),
                    ]

                def vprep(tb):
                    def emit():
                        for h in range(HPC):
                            hp = slice(h * HD, (h + 1) * HD)
                            vstage = vstage_pool.tile([128, 8, HD], F16, tag="vs")
                            nc.sync.dma_start_transpose(
                                vstage[:, :, :],
                                vt_sb[hp, tb * 1024:(tb + 1) * 1024],
                            )
                            vp = vp_pool.tile([128, 8, HD + 1], F16, tag="vp")
                            nc.gpsimd.memset(vp[:, :, HD:HD + 1], 1.0)
                            nc.vector.tensor_copy(vp[:, :, 0:HD], vstage[:, :, :])
                            vp_map[(tb, h)] = vp
                    return emit

                # state threaded between attention blocks
                pend_norm = [None]   # (cps pair, recf, cell for ctx_sb)
                pend_out = [None]    # (cell, toff, q0)
                dma_alt = [0]

                def block_tail(cps):
                    """rowsum extract + K=1 matmul broadcast + reciprocal,
                    off the next block's critical path."""
                    rrA = rr_pool.tile([1, QBLK], F32R, tag="rr")
                    nc.vector.tensor_copy(rrA[:, :], cps[0][HD:HD + 1, :])
                    rrB = rr_pool.tile([1, QBLK], F32R, tag="rr")
                    nc.vector.tensor_copy(rrB[:, :], cps[1][HD:HD + 1, :])
                    recb = mm_ps.tile([128, QBLK], F32, tag="mm")
                    for h, rr in ((0, rrA), (1, rrB)):
                        nc.tensor.matmul(
                            recb[:, :], ind_sbs[h][:, :], rr[:, :],
                            start=(h == 0), stop=(h == HPC - 1),
                        )
                    recf = rec_pool.tile([128, QBLK], F32, tag="recf")
                    nc.vector.reciprocal_approx_fast(recf[:, :], recb[:, :])
                    return recf

                def emit_norm():
                    cps, recf, cell = pend_norm[0]
                    pend_norm[0] = None
                    ctx_sb = norm_pool.tile([128, QBLK], F16, tag="ctx")
                    # fused psum-read + normalize + f16 cast
                    nc.vector.tensor_tensor(
                        ctx_sb[0:HD, :], cps[0][0:HD, :], recf[0:HD, :], MUL)
                    nc.vector.tensor_tensor(
                        ctx_sb[HD:128, :], cps[1][0:HD, :], recf[HD:128, :], MUL)
                    cell[0] = ctx_sb

                def outproj_half(tch, s0):
                    def emit():
                        cell, toff, q0 = pend_out[0]
                        ctx_sb = cell[0]
                        ops = mm_ps.tile([128, 512], F32, tag="mm")
                        nc.tensor.matmul(
                            ops[:, :],
                            ctx_sb[:, tch * 128:(tch + 1) * 128],
                            wo_sb[:, s0:s0 + 512],
                            start=True, stop=True,
                        )
                        osb = out_pool.tile([128, 512], F16, tag="o")
                        nc.vector.tensor_copy(osb[:, :], ops[:, :])
                        t0 = toff + q0 + tch * 128
                        if dma_alt[0] % 2 == 0:
                            nc.sync.dma_start(part[t0:t0 + 128, s0:s0 + 512],
                                              osb[:, :])
                        else:
                            nc.gpsimd.dma_start(part[t0:t0 + 128, s0:s0 + 512],
                                                osb[:, :])
                        dma_alt[0] += 1
                    return emit

                def emit_ctx(b, cps, pend_chunk, nk):
                    probs, c, off = pend_chunk
                    tbv = b * 2 + (c * 128) // 1024
                    for h in range(HPC):
                        src = vp_map[(tbv, h)][:, (c * 128 % 1024) // 128, :]
                        nc.tensor.matmul(
                            cps[h][:, off:QBLK], src, probs[:, h, off:QBLK],
                            start=(c == 0), stop=(c == nk - 1),
                        )

                def att_block(b, qb, fillers):
                    toff, q0 = b * S, qb * QBLK
                    nk = (q0 + QBLK) // KCH
                    cps0 = ctx_ps.tile([HD + 1, QBLK], F32, tag="c0")
                    cps1 = ctx_ps.tile([HD + 1, QBLK], F32, tag="c1")
                    cps = [cps0, cps1]
                    pend_chunk = None
                    fq = deque(fillers)
                    for c in range(nk):
                        off = max(0, 128 * c - q0)
                        sps = sps_ps.tile([128, 2, QBLK], F32, tag="sps")
                        for h in range(HPC):
                            hp = slice(h * HD, (h + 1) * HD)
                            nc.tensor.matmul(
                                sps[:, h, off:QBLK],
                                kt_sb[hp, toff + c * 128: toff + (c + 1) * 128],
                                qt_sb[hp, toff + q0 + off: toff + q0 + QBLK],
                                start=True, stop=True,
                            )
                        probs = probs_pool.tile([128, 2, QBLK], F16, tag="p")
                        nc.scalar.activation(
                            probs[:, :, off:], sps[:, :, off:], AF.Exp,
                            bias=0.0, scale=0.125,
                        )
                        if c * 128 >= q0:
                            nc.gpsimd.tensor_tensor(
                                probs[:, :, off:off + 128],
                                probs[:, :, off:off + 128],
                                tri_sb[:, :, :], MUL,
                            )
                        if c == 0:
                            # previous block's normalize: frees its ctx psum
                            # accumulators before this block's first ctx mm
                            if pend_norm[0] is not None:
                                emit_norm()
                        else:
                            if pend_chunk is not None:
                                emit_ctx(b, cps, pend_chunk, nk)
                            rem = nk - c
                            want = (len(fq) + rem - 1) // rem
                            for _ in range(min(want, len(fq))):
                                fq.popleft()()
                        pend_chunk = (probs, c, off)
                    emit_ctx(b, cps, pend_chunk, nk)
                    while fq:
                        fq.popleft()()
                    recf = block_tail(cps)
                    cell = [None]
                    pend_norm[0] = (cps, recf, cell)
                    pend_out[0] = (cell, toff, q0)

                # ================= emission =================
                g0 = qkv_groups(0)
                for g in (g0[0], g0[1], g0[4], g0[5]):   # q0, k0, v0, v512
                    g()
                vprep(0)()
                g0[2]()                                   # q512
                g0[3]()                                   # k512

                def outs():
                    return [outproj_half(tch, s0) for tch in range(QBLK // 128)
                            for s0 in (0, 512)]

                # fillers per (tb, block): QKV(tb+1) groups land one tb early;
                # QKV(tb3)'s v512+vprep slide into att(tb3) qb2 (its chunks
                # 0-7 only touch tb2 keys, so the deadline is chunk 8)
                carry = []
                for tb in range(NTB):
                    b = tb // 2
                    qbA = 2 * (tb % 2)
                    qbB = qbA + 1
                    if tb < NTB - 1:
                        nxt = qkv_groups(tb + 1)
                        if tb == NTB - 2:
                            fillA, fillB = nxt[:2], nxt[2:5]
                            nextcarry = [nxt[5], vprep(tb + 1)]
                        else:
                            fillA, fillB = nxt[:2], nxt[2:] + [vprep(tb + 1)]
                            nextcarry = []
                    else:
                        fillA, fillB, nextcarry = [], [], []
                    att_block(b, qbA, carry +
                              (outs() if pend_out[0] else []) + fillA)
                    att_block(b, qbB, outs() + fillB)
                    carry = nextcarry

                # trailing normalize + out-projection for the final block
                emit_norm()
                for tch in range(QBLK // 128):
                    for s0 in (0, 512):
                        outproj_half(tch, s0)()
    nc.compile()
    return nc


_NC_CACHE = None


def _get_nc():
    global _NC_CACHE
    if _NC_CACHE is None:
        _NC_CACHE = build_kernel()
    return _NC_CACHE


def _warr(w):
    """[D, CF] torch-style slice -> partition-major [128, NFC, 128]."""
    return np.ascontiguousarray(
        w.astype(np.float16).reshape(NFC, 128, CF).transpose(1, 0, 2)
    )


def make_in_maps(x, Wq, Wk, Wv, Wo):
    xf = x.reshape(T, D).astype(np.float16)
    xT = np.ascontiguousarray(
        xf.reshape(NTB, 1024, NFC, 128).transpose(3, 0, 2, 1)
    )
    tri1 = np.triu(np.ones((128, 128), dtype=np.float16))
    tri = np.ascontiguousarray(np.stack([tri1, tri1], axis=1))
    ind2 = np.zeros((2, 128), dtype=np.float32)
    ind2[0, 0:64] = 1.0
    ind2[1, 64:128] = 1.0
    in_maps = []
    for c in range(NCORES):
        rs = slice(c * CF, (c + 1) * CF)
        in_maps.append({
            "xT": xT,
            "wq": _warr(Wq[rs, :].T),
            "wk": _warr(Wk[rs, :].T),
            "wv": _warr(Wv[rs, :].T),
            "wo": np.ascontiguousarray(Wo[:, rs].T.astype(np.float16)),
            "tri": tri,
            "ind2": ind2,
        })
    return in_maps


def kernel(x, Wq, Wk, Wv, Wo, bo):
    x = np.asarray(x, dtype=np.float32)
    Wq = np.asarray(Wq, dtype=np.float32)
    Wk = np.asarray(Wk, dtype=np.float32)
    Wv = np.asarray(Wv, dtype=np.float32)
    Wo = np.asarray(Wo, dtype=np.float32)
    bo = np.asarray(bo, dtype=np.float32)

    in_maps = make_in_maps(x, Wq, Wk, Wv, Wo)
    res = run_bass_kernel_spmd(_get_nc(), in_maps, core_ids=list(range(NCORES)))
    out = res.results[0]["part"].astype(np.float32)
    for c in range(1, NCORES):
        out += res.results[c]["part"].astype(np.float32)
    out += bo[None, :]
    return out.reshape(B, S, D)
